# revision 12
# baseline (speedup 1.0000x reference)
"""Trainium2 Bass kernel for nn_AttentionBlock (GroupNorm -> MHA -> proj + residual).

Contract: kernel(**inputs) takes the FULL unsharded inputs (as produced by
setup_inputs) and returns the FULL output [8, 512, 32, 32] float32.

Sharding: pure data-parallel over batch B=8 across the 8 NeuronCores; each core
processes one batch element end-to-end (no collectives needed).

Per-core layout / algorithm (B=1, C=512, N=H*W=1024, heads=8, head_dim=64):
  - All matmuls bf16 (fp8 DoubleRow was measured to trigger a hardware
    power throttle to half clock, netting ~0 gain while slowing neighbors).
  - GroupNorm(32 groups) in fp32; bn_stats/bn_aggr per 128-channel tile,
    batched group-combine + broadcast via tiny PE matmuls, DVE-only rsqrt.
    All elementwise on DVE (gpsimd tensor ops measured at ~14.7us per
    [128,1024] tile -- unusable).
  - Attention in "S^T" layout: S^T[m,n] = sum_c k[c,m] q[c,n], K=64 bf16.
    exp on ScalarE -> bf16 E tiles ([128,2,512], 64 ACTIVATEs).
    AV with lhsT = [ones(64) | v(64)] per head: PSUM rows 0:64 hold the
    softmax denominator broadcast (partition offset 0 so
    reciprocal_approx_fast can read it directly), rows 64:128 hold O.
  - Blocks are half-major: b=0..7 -> (pr=b%4, half=b//4); O accumulates per
    block in one PSUM tile [128, 2(hi), 512]. Epilogue = 1 reciprocal +
    2 tensor_tensor mults straight out of PSUM (no copies).
  - proj per (r, half) split kc 0:2 / 2:4; first half fuses x + pb via
    scalar_tensor_tensor, second half adds and streams the output DMA
    inside the attention stream (only the last half=1 quarter is tail).
  - v-bias folded into pb on host (pb_eff = proj_b + proj_w @ b_v); q scale
    folded into wq/bq.
  - Static drip schedule interleaves qkv/vt/proj matmuls into the attention
    stream keyed on exp-tile index; AV lags exp by LAG units (software
    pipeline) so the PE never waits on ScalarE.
"""

import numpy as np
import ml_dtypes

import concourse.bass as bass
import concourse.tile as tile
from concourse import bacc, mybir
from concourse.bass_utils import run_bass_kernel_spmd

FP32 = mybir.dt.float32
BF16 = mybir.dt.bfloat16
AF = mybir.ActivationFunctionType
OP = mybir.AluOpType

P = 128      # SBUF partitions
C = 512      # channels
NT = 1024    # spatial tokens (32*32)
CT = C // P  # channel tiles = 4
MT = NT // P # key tiles = 8
NH = 8       # heads
HD = 64      # head dim
NCORES = 8
GSZ = 16     # channels per group (512/32)

LAG = 8  # AV units behind exp


def _emit(tc: "tile.TileContext", io: dict):
    nc = tc.nc
    from collections import deque
    import contextlib
    ctx = contextlib.ExitStack()
    with ctx:
        pers = ctx.enter_context(tc.tile_pool(name="pers", bufs=1))
        sm = ctx.enter_context(tc.tile_pool(name="small", bufs=1))

        x, wq, wk, pw = io["x"], io["wq"], io["wk"], io["pw"]
        out = io["out"]

        # ---------------- input DMAs ----------------
        # x split into 8 half-tile chunks balanced over the 3 DMA-capable
        # queues (sync / gpsimd / scalar), ahead of the weights: per-queue
        # bandwidth is ~150 GB/s, and GroupNorm's bn_stats are per half-tile
        # so they pipeline with the chunks.
        x_r = x.rearrange("(r p) n -> p r n", p=P)
        x_sb = pers.tile([P, CT, NT], FP32, tag="x")
        qs = [nc.sync, nc.gpsimd, nc.scalar]
        ci = 0
        for r in range(CT):
            for hf in range(2):
                qs[ci % 3].dma_start(x_sb[:, r, 512 * hf:512 * hf + 512],
                                     x_r[:, r, 512 * hf:512 * hf + 512])
                ci += 1
        # small tensors on the scalar queue
        amat_sb = pers.tile([P, NH], FP32, tag="amat")
        nc.scalar.dma_start(amat_sb, io["amat"])
        imat_sb = pers.tile([NH, P], FP32, tag="imat")
        nc.scalar.dma_start(imat_sb, io["imat"])
        gg_sb = pers.tile([P, CT], FP32, tag="gg")
        nc.scalar.dma_start(gg_sb, io["gg"].rearrange("(r p) -> p r", p=P))
        gb_sb = pers.tile([P, CT], FP32, tag="gb")
        nc.scalar.dma_start(gb_sb, io["gb"].rearrange("(r p) -> p r", p=P))
        bq_sb = pers.tile([P, CT], FP32, tag="bq")
        nc.scalar.dma_start(bq_sb, io["bq"].rearrange("(r p) -> p r", p=P))
        bk_sb = pers.tile([P, CT], FP32, tag="bk")
        nc.scalar.dma_start(bk_sb, io["bk"].rearrange("(r p) -> p r", p=P))
        pb_sb = pers.tile([P, CT], FP32, tag="pb")
        nc.scalar.dma_start(pb_sb, io["pb"].rearrange("(r p) -> p r", p=P))
        # weights after x: wk/wq first (needed ~15us in), v fp8 (accuracy
        # ablation shows fp8 wv is benign), pw late (first use ~40us in)
        wk_sb = pers.tile([P, CT, C], BF16, tag="wk")
        nc.gpsimd.dma_start(wk_sb, wk)
        wq_sb = pers.tile([P, CT, C], BF16, tag="wq")
        nc.sync.dma_start(wq_sb, wq)
        wv_sb = pers.tile([P, CT, C], mybir.dt.float8e4, tag="wv")
        nc.scalar.dma_start(wv_sb, io["wv8"])
        pw_sb = pers.tile([P, CT, C], BF16, tag="pw")
        nc.sync.dma_start(pw_sb, pw)

        # preload the exp activation table while DMAs are in flight
        warm_sb = pers.tile([1, 1], FP32, tag="actwarm")
        nc.vector.memset(warm_sb, 0.0)
        nc.scalar.activation(warm_sb, warm_sb, AF.Exp)

        # persistent SBUF
        h_sb = pers.tile([P, CT, NT], BF16, tag="h")
        q_sb = pers.tile([P, CT, NT], BF16, tag="q")
        k_sb = pers.tile([P, CT, NT], BF16, tag="k")
        # vT per head block: cols 0:64 = ones (denominator), 64:128 = v
        vT_sb = pers.tile([P, MT, NH * P], BF16, tag="vT")
        O_sb = pers.tile([P, CT, NT], BF16, tag="O")
        P1x_sb = pers.tile([P, CT, NT], FP32, tag="p1x")

        nc.gpsimd.memset(
            vT_sb.rearrange("p t (h c) -> p t h c", c=P)[:, :, :, 0:HD], 1.0)

        # ---------------- GroupNorm ----------------
        with nc.named_scope("gn"), \
             tc.tile_pool(name="gnps", bufs=1, space="PSUM") as gnps, \
             tc.tile_pool(name="mrps", bufs=1, space="PSUM") as mrps:
            st2_all = sm.tile([P, CT, 2], FP32, tag="st2_all")
            mv_all = sm.tile([P, CT, 2], FP32, tag="mv_all")
            for r in range(CT):
                st = sm.tile([P, 2, 6], FP32, tag=f"bnstats{r}")
                nc.vector.bn_stats(st[:, 0, :], x_sb[:, r, 0:512])
                nc.vector.bn_stats(st[:, 1, :], x_sb[:, r, 512:1024])
                nc.vector.bn_aggr(mv_all[:, r, :], st)
            nc.vector.tensor_copy(st2_all[:, :, 0:1], mv_all[:, :, 0:1])
            nc.vector.tensor_tensor(st2_all[:, :, 1:2], mv_all[:, :, 0:1],
                                    mv_all[:, :, 0:1], OP.mult)
            nc.vector.tensor_tensor(st2_all[:, :, 1:2], st2_all[:, :, 1:2],
                                    mv_all[:, :, 1:2], OP.add)
            G_ps = gnps.tile([NH, CT, 2], FP32, tag="gps")
            nc.tensor.matmul(G_ps, amat_sb,
                             st2_all.rearrange("p r k -> p (r k)"),
                             start=True, stop=True)
            st_all = sm.tile([NH, CT, 2], FP32, tag="st_all")
            nc.vector.tensor_copy(st_all, G_ps)
            var_all = sm.tile([NH, CT], FP32, tag="var_all")
            nc.vector.tensor_tensor(var_all[:, :, None], st_all[:, :, 0:1],
                                    st_all[:, :, 0:1], OP.mult)
            nc.vector.tensor_tensor(var_all[:, :, None], st_all[:, :, 1:2],
                                    var_all[:, :, None], OP.subtract)
            # rstd = sqrt(1/(var + eps)): fast DVE reciprocal + ScalarE sqrt
            nc.vector.tensor_scalar(var_all, var_all, 1e-5, None, OP.add)
            y = sm.tile([NH, CT], FP32, tag="rsqrt_y")
            nc.vector.reciprocal_approx_fast(y, var_all)
            nc.scalar.activation(st_all[:, :, 1:2], y[:, :, None], AF.Sqrt)
            MR_ps = mrps.tile([P, CT, 2], FP32, tag="mrps")
            nc.tensor.matmul(MR_ps, imat_sb,
                             st_all.rearrange("p r k -> p (r k)"),
                             start=True, stop=True)
            mr = sm.tile([P, CT, 2], FP32, tag="mr")
            nc.vector.tensor_copy(mr, MR_ps)
            a_all = sm.tile([P, CT, 1], FP32, tag="gn_a")
            nc.vector.tensor_tensor(a_all, mr[:, :, 1:2], gg_sb[:, :, None],
                                    OP.mult)
            b_all = sm.tile([P, CT, 1], FP32, tag="gn_b")
            nc.vector.tensor_tensor(b_all, mr[:, :, 0:1], a_all, OP.mult)
            nc.vector.tensor_tensor(b_all, gb_sb[:, :, None], b_all,
                                    OP.subtract)
            # normalize split across ScalarE (Identity with per-partition
            # scale/bias APs) and DVE so h is ready ~2x sooner
            for r in range(CT):
                if r % 2 == 0:
                    nc.scalar.activation(h_sb[:, r, :], x_sb[:, r, :],
                                         AF.Identity, bias=b_all[:, r, :],
                                         scale=a_all[:, r, :])
                else:
                    nc.vector.tensor_scalar(h_sb[:, r, :], x_sb[:, r, :],
                                            a_all[:, r, :], b_all[:, r, :],
                                            OP.mult, OP.add)

        # ------------- qkv + attention -------------
        with nc.named_scope("qkv_attn"), \
             tc.tile_pool(name="spool", bufs=1, space="PSUM") as spool, \
             tc.tile_pool(name="opool", bufs=1, space="PSUM") as opool, \
             tc.tile_pool(name="bgps", bufs=1, space="PSUM") as bgps, \
             tc.tile_pool(name="epool", bufs=8) as epool, \
             tc.tile_pool(name="rpool", bufs=2) as rpool, \
             tc.tile_pool(name="outp", bufs=4) as outp:

            out_r = out.rearrange("(r p) n -> p r n", p=P)

            def bg_tile(name):
                return bgps.tile([P, 512], FP32, tag="bg", name=name)

            def qk_task(dst, w_sb, b_sb, r, half, on_scalar=False):
                ps = bg_tile(f"qk_{r}_{half}_{w_sb.name}")
                for kc in range(CT):
                    nc.tensor.matmul(
                        ps, w_sb[:, kc, P * r:P * r + P],
                        h_sb[:, kc, 512 * half:512 * half + 512],
                        start=(kc == 0), stop=(kc == CT - 1))
                dst_ap = dst[:, r, 512 * half:512 * half + 512]
                if on_scalar:
                    nc.scalar.add(dst_ap, ps, b_sb[:, r:r + 1])
                else:
                    nc.vector.tensor_scalar(dst_ap, ps, b_sb[:, r:r + 1],
                                            None, OP.add)

            def vt_task(t):
                ps = bg_tile(f"vt{t}")
                for kc in range(CT):
                    nc.tensor.matmul(ps, h_sb[:, kc, P * t:P * t + P],
                                     wv_sb[:, kc, :],
                                     start=(kc == 0), stop=(kc == CT - 1))
                nc.vector.tensor_copy(
                    vT_sb[:, t, :].rearrange("p (h c) -> p h c",
                                             c=P)[:, :, HD:P],
                    ps.rearrange("p (h c) -> p h c", c=HD))

            def proj01_task(r, half):
                hs = 512 * half
                ps = bg_tile(f"pjA_{r}_{half}")
                for kc in range(2):
                    nc.tensor.matmul(ps, pw_sb[:, kc, P * r:P * r + P],
                                     O_sb[:, kc, hs:hs + 512],
                                     start=(kc == 0), stop=(kc == 1))
                # P1x = (ps + pb) + x in one fused DVE op
                nc.vector.scalar_tensor_tensor(
                    P1x_sb[:, r, hs:hs + 512], ps, pb_sb[:, r:r + 1],
                    x_sb[:, r, hs:hs + 512], OP.add, OP.add)

            def proj23_task(r, half):
                hs = 512 * half
                ps = bg_tile(f"pjB_{r}_{half}")
                for kc in range(2, 4):
                    nc.tensor.matmul(ps, pw_sb[:, kc, P * r:P * r + P],
                                     O_sb[:, kc, hs:hs + 512],
                                     start=(kc == 2), stop=(kc == 3))
                o_st = outp.tile([P, 512], FP32, tag="ost",
                                 name=f"ost{r}_{half}")
                nc.vector.tensor_tensor(o_st, ps,
                                        P1x_sb[:, r, hs:hs + 512], OP.add)
                eng = nc.sync if (r + half) % 2 == 0 else nc.gpsimd
                eng.dma_start(out_r[:, r, hs:hs + 512], o_st)

            # upfront: deps of block 0 (pr0, half0); copies ride ScalarE
            qk_task(k_sb, wk_sb, bk_sb, 0, 0, on_scalar=True)
            qk_task(k_sb, wk_sb, bk_sb, 0, 1, on_scalar=True)
            qk_task(q_sb, wq_sb, bq_sb, 0, 0, on_scalar=True)

            # drip schedule: exp-tile index (0..63) -> tasks. blocks are
            # half-major: b = 0..7 -> (pr = b % 4, half = b // 4); epilogue
            # of block b is emitted around tile 8b + 8 + LAG/2.
            drip = {
                0: [(vt_task, (0,)), (vt_task, (1,))],
                1: [(vt_task, (2,)), (vt_task, (3,))],
                2: [(vt_task, (4,)), (vt_task, (5,))],
                3: [(vt_task, (6,)), (vt_task, (7,))],
                4: [(qk_task, (k_sb, wk_sb, bk_sb, 1, 0))],
                5: [(qk_task, (k_sb, wk_sb, bk_sb, 1, 1))],
                6: [(qk_task, (q_sb, wq_sb, bq_sb, 1, 0))],
                9: [(qk_task, (k_sb, wk_sb, bk_sb, 2, 0))],
                11: [(qk_task, (k_sb, wk_sb, bk_sb, 2, 1))],
                13: [(qk_task, (q_sb, wq_sb, bq_sb, 2, 0))],
                17: [(qk_task, (k_sb, wk_sb, bk_sb, 3, 0))],
                19: [(qk_task, (k_sb, wk_sb, bk_sb, 3, 1))],
                21: [(qk_task, (q_sb, wq_sb, bq_sb, 3, 0))],
                25: [(qk_task, (q_sb, wq_sb, bq_sb, 0, 1))],
                20: [(proj01_task, (0, 0))],
                22: [(proj01_task, (1, 0))],
                24: [(proj01_task, (2, 0))],
                26: [(proj01_task, (3, 0))],
                33: [(qk_task, (q_sb, wq_sb, bq_sb, 1, 1))],
                37: [(proj23_task, (0, 0))],
                39: [(proj23_task, (1, 0))],
                41: [(proj23_task, (2, 0))],
                43: [(proj23_task, (3, 0))],
                45: [(qk_task, (q_sb, wq_sb, bq_sb, 2, 1))],
                49: [(qk_task, (q_sb, wq_sb, bq_sb, 3, 1))],
                53: [(proj01_task, (0, 1))],
                55: [(proj01_task, (1, 1))],
                57: [(proj01_task, (2, 1))],
                59: [(proj01_task, (3, 1))],
            }

            # per-hi O accumulators: [128, 512] = 1 PSUM bank each, 3
            # rotating bufs so the next block's AV never waits on the
            # previous block's epilogue
            O_cur = {}
            o_i = [0]

            def emit_av(b, hi, t, E_t, j):
                pr, half = b % 4, b // 4
                if t == 0:
                    o_i[0] = (o_i[0] + 1) % 3
                    O_cur[hi] = opool.tile([P, 512], FP32,
                                           tag=f"o{o_i[0]}",
                                           name=f"o{b}_{hi}")
                h = 2 * pr + hi
                nc.tensor.matmul(
                    O_cur[hi],
                    vT_sb[:, t, P * h:P * h + P],
                    E_t[:, j, :],
                    start=(t == 0), stop=(t == MT - 1))

            def emit_epilogue(b, hi):
                pr, half = b % 4, b // 4
                hs = 512 * half
                O_hi = O_cur.pop(hi)
                Rh = rpool.tile([HD, 512], FP32, tag="rh", name=f"rh{b}_{hi}")
                # D is broadcast on PSUM rows 0:64 (partition offset 0, as
                # the fast reciprocal requires)
                nc.vector.reciprocal_approx_fast(Rh, O_hi[0:HD, :])
                nc.vector.tensor_tensor(
                    O_sb[HD * hi:HD * hi + HD, pr, hs:hs + 512],
                    O_hi[HD:P, :], Rh, OP.mult)

            pend = deque()

            def flush_unit():
                b, hi, t, E_t, j = pend.popleft()
                emit_av(b, hi, t, E_t, j)
                if t == MT - 1:
                    emit_epilogue(b, hi)

            s_i = [0]
            for ti in range(64):
                b, rem = ti // 8, ti % 8
                pr, half = b % 4, b // 4
                hi, u = rem // 4, rem % 4
                s_i[0] ^= 1
                S_t = spool.tile([P, 2, 512], FP32, tag=f"s{s_i[0]}",
                                 name=f"st{ti}")
                for j in range(2):
                    t = 2 * u + j
                    nc.tensor.matmul(
                        S_t[:, j, :],
                        k_sb[HD * hi:HD * hi + HD, pr, P * t:P * t + P],
                        q_sb[HD * hi:HD * hi + HD, pr,
                             512 * half:512 * half + 512],
                        start=True, stop=True)
                E_t = epool.tile([P, 2, 512], BF16, tag="e", name=f"et{ti}")
                nc.scalar.activation(E_t, S_t, AF.Exp)
                for j in range(2):
                    pend.append((b, hi, 2 * u + j, E_t, j))
                while len(pend) > LAG:
                    flush_unit()
                for fn, args in drip.pop(ti, ()):
                    fn(*args)
            while pend:
                flush_unit()
            assert not drip, f"undripped: {list(drip)}"

            # ---------------- tail: proj kc 2:4 for half 1 ----------------
            with nc.named_scope("proj_tail"):
                for r in range(CT):
                    proj23_task(r, 1)


_CACHE: dict = {}


def _build():
    if "nc" in _CACHE:
        return _CACHE["nc"]
    nc = bacc.Bacc("TRN2", target_bir_lowering=False, debug=False,
                   num_devices=NCORES)
    io = {
        "x": nc.dram_tensor("x", [C, NT], FP32, kind="ExternalInput").ap(),
        "wq": nc.dram_tensor("wq", [P, CT, C], BF16, kind="ExternalInput").ap(),
        "wk": nc.dram_tensor("wk", [P, CT, C], BF16, kind="ExternalInput").ap(),
        "wv8": nc.dram_tensor("wv8", [P, CT, C], mybir.dt.float8e4,
                              kind="ExternalInput").ap(),
        "pw": nc.dram_tensor("pw", [P, CT, C], BF16, kind="ExternalInput").ap(),
        "bq": nc.dram_tensor("bq", [C], FP32, kind="ExternalInput").ap(),
        "bk": nc.dram_tensor("bk", [C], FP32, kind="ExternalInput").ap(),
        "pb": nc.dram_tensor("pb", [C], FP32, kind="ExternalInput").ap(),
        "gg": nc.dram_tensor("gg", [C], FP32, kind="ExternalInput").ap(),
        "gb": nc.dram_tensor("gb", [C], FP32, kind="ExternalInput").ap(),
        "amat": nc.dram_tensor("amat", [P, NH], FP32, kind="ExternalInput").ap(),
        "imat": nc.dram_tensor("imat", [NH, P], FP32, kind="ExternalInput").ap(),
        "out": nc.dram_tensor("out", [C, NT], FP32, kind="ExternalOutput").ap(),
    }
    with tile.TileContext(nc) as tc:
        _emit(tc, io)
    nc.compile()
    _CACHE["nc"] = nc
    return nc


def _host_prep(inputs):
    x = np.ascontiguousarray(np.asarray(inputs["x"], dtype=np.float32))
    qkv_w = np.asarray(inputs["qkv_w"], dtype=np.float32)
    qkv_b = np.asarray(inputs["qkv_b"], dtype=np.float32)
    proj_w = np.asarray(inputs["proj_w"], dtype=np.float32)
    proj_b = np.asarray(inputs["proj_b"], dtype=np.float32)
    gn_scale = np.asarray(inputs["gn_scale"], dtype=np.float32)
    gn_bias = np.asarray(inputs["gn_bias"], dtype=np.float32)

    s = np.float32(1.0 / np.sqrt(HD))
    bf = ml_dtypes.bfloat16

    def pack_qk(w):
        # [p, kc, oc] = w[oc, 128*kc + p]
        return np.ascontiguousarray(
            w.reshape(C, CT, P).transpose(2, 1, 0)).astype(bf)

    shared = {
        "wq": pack_qk(qkv_w[0:C] * s),
        "wk": pack_qk(qkv_w[C:2 * C]),
        "wv8": np.ascontiguousarray(
            qkv_w[2 * C:3 * C].reshape(C, CT, P).transpose(2, 1, 0)
        ).astype(ml_dtypes.float8_e4m3),
        "pw": pack_qk(proj_w),
        "bq": (qkv_b[0:C] * s).astype(np.float32),
        "bk": qkv_b[C:2 * C].astype(np.float32),
        # v bias and proj bias folded: proj(o + b_v) = proj(o) + W_p b_v
        "pb": (proj_b + proj_w @ qkv_b[2 * C:3 * C]).astype(np.float32),
        "gg": gn_scale,
        "gb": gn_bias,
        "amat": (np.kron(np.eye(NH, dtype=np.float32),
                         np.ones((GSZ, 1), np.float32)) / GSZ),
        "imat": np.ascontiguousarray(np.kron(np.eye(NH, dtype=np.float32),
                                             np.ones((1, GSZ), np.float32))),
    }
    B = x.shape[0]
    in_maps = []
    for b in range(B):
        m = dict(shared)
        m["x"] = np.ascontiguousarray(x[b].reshape(C, NT))
        in_maps.append(m)
    return in_maps


def run(inputs, trace=False):
    nc = _build()
    in_maps = _host_prep(inputs)
    res = run_bass_kernel_spmd(nc, in_maps, list(range(NCORES)), trace=trace)
    out = np.stack([res.results[i]["out"] for i in range(NCORES)], axis=0)
    return out.reshape(len(in_maps), C, 32, 32), res


def kernel(**inputs) -> np.ndarray:
    out, _ = run(inputs, trace=False)
    return out.astype(np.float32)


# revision 18
# speedup vs baseline: 1.0300x; 1.0300x over previous
"""Trainium2 Bass kernel for nn_AttentionBlock (GroupNorm -> MHA -> proj + residual).

Contract: kernel(**inputs) takes the FULL unsharded inputs (as produced by
setup_inputs) and returns the FULL output [8, 512, 32, 32] float32.

Sharding: pure data-parallel over batch B=8 across the 8 NeuronCores; each core
processes one batch element end-to-end (no collectives needed).

Per-core layout / algorithm (B=1, C=512, N=H*W=1024, heads=8, head_dim=64):
  - All matmuls bf16 (fp8 DoubleRow was measured to trigger a hardware
    power throttle to half clock, netting ~0 gain while slowing neighbors).
  - GroupNorm(32 groups) in fp32; bn_stats/bn_aggr per 128-channel tile,
    batched group-combine + broadcast via tiny PE matmuls, DVE-only rsqrt.
    All elementwise on DVE (gpsimd tensor ops measured at ~14.7us per
    [128,1024] tile -- unusable).
  - Attention in "S^T" layout: S^T[m,n] = sum_c k[c,m] q[c,n], K=64 bf16.
    exp on ScalarE -> bf16 E tiles ([128,2,512], 64 ACTIVATEs).
    AV with lhsT = [ones(64) | v(64)] per head: PSUM rows 0:64 hold the
    softmax denominator broadcast (partition offset 0 so
    reciprocal_approx_fast can read it directly), rows 64:128 hold O.
  - Blocks are half-major: b=0..7 -> (pr=b%4, half=b//4); O accumulates per
    block in one PSUM tile [128, 2(hi), 512]. Epilogue = 1 reciprocal +
    2 tensor_tensor mults straight out of PSUM (no copies).
  - proj per (r, half) split kc 0:2 / 2:4; first half fuses x + pb via
    scalar_tensor_tensor, second half adds and streams the output DMA
    inside the attention stream (only the last half=1 quarter is tail).
  - v-bias folded into pb on host (pb_eff = proj_b + proj_w @ b_v); q scale
    folded into wq/bq.
  - Static drip schedule interleaves qkv/vt/proj matmuls into the attention
    stream keyed on exp-tile index; AV lags exp by LAG units (software
    pipeline) so the PE never waits on ScalarE.
"""

import numpy as np
import ml_dtypes

import concourse.bass as bass
import concourse.tile as tile
from concourse import bacc, mybir
from concourse.bass_utils import run_bass_kernel_spmd

FP32 = mybir.dt.float32
BF16 = mybir.dt.bfloat16
AF = mybir.ActivationFunctionType
OP = mybir.AluOpType

P = 128      # SBUF partitions
C = 512      # channels
NT = 1024    # spatial tokens (32*32)
CT = C // P  # channel tiles = 4
MT = NT // P # key tiles = 8
NH = 8       # heads
HD = 64      # head dim
NCORES = 8
GSZ = 16     # channels per group (512/32)

LAG = 8  # AV units behind exp


def _emit(tc: "tile.TileContext", io: dict):
    nc = tc.nc
    from collections import deque
    import contextlib
    ctx = contextlib.ExitStack()
    with ctx:
        pers = ctx.enter_context(tc.tile_pool(name="pers", bufs=1))
        sm = ctx.enter_context(tc.tile_pool(name="small", bufs=1))

        x, wq, wk, pw = io["x"], io["wq"], io["wk"], io["pw"]
        out = io["out"]

        # ---------------- input DMAs ----------------
        # ~150 GB/s per queue; x (2MB) + wq/wk (1MB) must all land before the
        # first S matmul, so balance ~1MB per queue and split wq/wk by
        # kc-halves so early contraction steps don't wait for the full
        # tensor. pw/pb ride late (first use ~40us in).
        x_r = x.rearrange("(r p) n -> p r n", p=P)
        x_sb = pers.tile([P, CT, NT], FP32, tag="x")
        wq_sb = pers.tile([P, CT, C], BF16, tag="wq")
        wk_sb = pers.tile([P, CT, C], BF16, tag="wk")
        wv_sb = pers.tile([P, CT, C], mybir.dt.float8e4, tag="wv")
        pw_sb = pers.tile([P, CT, C], BF16, tag="pw")

        def xch(r, hf):
            return x_sb[:, r, 512 * hf:512 * hf + 512], \
                   x_r[:, r, 512 * hf:512 * hf + 512]

        # sync: small GN tensors, then x tile 0, wq, bias smalls, pw
        amat_sb = pers.tile([P, NH], FP32, tag="amat")
        nc.sync.dma_start(amat_sb, io["amat"])
        imat_sb = pers.tile([NH, P], FP32, tag="imat")
        nc.sync.dma_start(imat_sb, io["imat"])
        gg_sb = pers.tile([P, CT], FP32, tag="gg")
        nc.sync.dma_start(gg_sb, io["gg"].rearrange("(r p) -> p r", p=P))
        gb_sb = pers.tile([P, CT], FP32, tag="gb")
        nc.sync.dma_start(gb_sb, io["gb"].rearrange("(r p) -> p r", p=P))
        nc.sync.dma_start(*xch(0, 0))
        nc.sync.dma_start(*xch(0, 1))
        nc.sync.dma_start(wq_sb[:, 0:2, :], wq[:, 0:2, :])
        nc.sync.dma_start(wq_sb[:, 2:4, :], wq[:, 2:4, :])
        bq_sb = pers.tile([P, CT], FP32, tag="bq")
        nc.sync.dma_start(bq_sb, io["bq"].rearrange("(r p) -> p r", p=P))
        bk_sb = pers.tile([P, CT], FP32, tag="bk")
        nc.sync.dma_start(bk_sb, io["bk"].rearrange("(r p) -> p r", p=P))
        pb_sb = pers.tile([P, CT], FP32, tag="pb")
        nc.sync.dma_start(pb_sb, io["pb"].rearrange("(r p) -> p r", p=P))
        nc.sync.dma_start(pw_sb, pw)
        # gpsimd: x tile 1, wk halves
        nc.gpsimd.dma_start(*xch(1, 0))
        nc.gpsimd.dma_start(*xch(1, 1))
        nc.gpsimd.dma_start(wk_sb[:, 0:2, :], wk[:, 0:2, :])
        nc.gpsimd.dma_start(wk_sb[:, 2:4, :], wk[:, 2:4, :])
        # scalar: x tiles 2 and 3, wv fp8 (accuracy ablation: fp8 wv benign)
        nc.scalar.dma_start(*xch(2, 0))
        nc.scalar.dma_start(*xch(2, 1))
        nc.scalar.dma_start(*xch(3, 0))
        nc.scalar.dma_start(*xch(3, 1))
        nc.scalar.dma_start(wv_sb, io["wv8"])

        # preload the exp activation table while DMAs are in flight
        warm_sb = pers.tile([1, 1], FP32, tag="actwarm")
        nc.vector.memset(warm_sb, 0.0)
        nc.scalar.activation(warm_sb, warm_sb, AF.Exp)

        # persistent SBUF
        h_sb = pers.tile([P, CT, NT], BF16, tag="h")
        q_sb = pers.tile([P, CT, NT], BF16, tag="q")
        k_sb = pers.tile([P, CT, NT], BF16, tag="k")
        # vT per head block: cols 0:64 = ones (denominator), 64:128 = v
        vT_sb = pers.tile([P, MT, NH * P], BF16, tag="vT")
        O_sb = pers.tile([P, CT, NT], BF16, tag="O")
        P1x_sb = pers.tile([P, CT, NT], FP32, tag="p1x")

        nc.gpsimd.memset(
            vT_sb.rearrange("p t (h c) -> p t h c", c=P)[:, :, :, 0:HD], 1.0)

        # ---------------- GroupNorm (per-tile pipelined) ----------------
        # groups (16ch) never cross a 128-channel tile, so each tile's full
        # GN chain runs as soon as its x chunks land, overlapped with later
        # DMAs; the three upfront q/k accumulations consume h tiles as they
        # appear (each in its own head-scoped PSUM bank).
        with nc.named_scope("gn"), \
             tc.tile_pool(name="gnps", bufs=1, space="PSUM") as gnps, \
             tc.tile_pool(name="mrps", bufs=1, space="PSUM") as mrps, \
             tc.tile_pool(name="hqk", bufs=1, space="PSUM") as hqk:
            up_ps = [hqk.tile([P, 512], FP32, tag=f"up{i}", name=f"up{i}")
                     for i in range(3)]
            up_spec = [(wk_sb, 0), (wk_sb, 1), (wq_sb, 0)]
            for r in range(CT):
                st = sm.tile([P, 2, 6], FP32, tag=f"bnstats{r}")
                nc.vector.bn_stats(st[:, 0, :], x_sb[:, r, 0:512])
                nc.vector.bn_stats(st[:, 1, :], x_sb[:, r, 512:1024])
                mv = sm.tile([P, 2], FP32, tag=f"mv{r}")
                nc.vector.bn_aggr(mv, st)
                st2 = sm.tile([P, 2], FP32, tag=f"st2_{r}")
                nc.vector.tensor_copy(st2[:, 0:1], mv[:, 0:1])
                nc.vector.tensor_tensor(st2[:, 1:2], mv[:, 0:1], mv[:, 0:1],
                                        OP.mult)
                nc.vector.tensor_tensor(st2[:, 1:2], st2[:, 1:2], mv[:, 1:2],
                                        OP.add)
                G = gnps.tile([NH, 2], FP32, tag="g", name=f"g{r}")
                nc.tensor.matmul(G, amat_sb, st2, start=True, stop=True)
                stg = sm.tile([NH, 2], FP32, tag=f"stg{r}")
                nc.vector.tensor_copy(stg, G)
                var = sm.tile([NH, 1], FP32, tag=f"var{r}")
                nc.vector.tensor_tensor(var, stg[:, 0:1], stg[:, 0:1],
                                        OP.mult)
                nc.vector.tensor_tensor(var, stg[:, 1:2], var, OP.subtract)
                nc.vector.tensor_scalar(var, var, 1e-5, None, OP.add)
                # rstd = sqrt(1/(var+eps)): fast DVE reciprocal + ScalarE sqrt
                yv = sm.tile([NH, 1], FP32, tag=f"y{r}")
                nc.vector.reciprocal_approx_fast(yv, var)
                nc.scalar.activation(stg[:, 1:2], yv, AF.Sqrt)
                MR = mrps.tile([P, 2], FP32, tag="mr", name=f"mr{r}")
                nc.tensor.matmul(MR, imat_sb, stg, start=True, stop=True)
                a_r = sm.tile([P, 1], FP32, tag=f"a{r}")
                nc.vector.tensor_tensor(a_r, MR[:, 1:2], gg_sb[:, r:r + 1],
                                        OP.mult)
                b_r = sm.tile([P, 1], FP32, tag=f"b{r}")
                nc.vector.tensor_tensor(b_r, MR[:, 0:1], a_r, OP.mult)
                nc.vector.tensor_tensor(b_r, gb_sb[:, r:r + 1], b_r,
                                        OP.subtract)
                if r % 2 == 0:
                    nc.scalar.activation(h_sb[:, r, :], x_sb[:, r, :],
                                         AF.Identity, bias=b_r, scale=a_r)
                else:
                    nc.vector.tensor_scalar(h_sb[:, r, :], x_sb[:, r, :],
                                            a_r, b_r, OP.mult, OP.add)
                # feed this h tile into the three upfront q/k accumulations
                for i, (w_sb, half) in enumerate(up_spec):
                    nc.tensor.matmul(
                        up_ps[i], w_sb[:, r, 0:P],
                        h_sb[:, r, 512 * half:512 * half + 512],
                        start=(r == 0), stop=(r == CT - 1))
            # bias copies on ScalarE (idle until the exp stream starts)
            nc.scalar.add(k_sb[:, 0, 0:512], up_ps[0], bk_sb[:, 0:1])
            nc.scalar.add(k_sb[:, 0, 512:1024], up_ps[1], bk_sb[:, 0:1])
            nc.scalar.add(q_sb[:, 0, 0:512], up_ps[2], bq_sb[:, 0:1])

        # ------------- qkv + attention -------------
        with nc.named_scope("qkv_attn"), \
             tc.tile_pool(name="spool", bufs=1, space="PSUM") as spool, \
             tc.tile_pool(name="opool", bufs=1, space="PSUM") as opool, \
             tc.tile_pool(name="bgps", bufs=1, space="PSUM") as bgps, \
             tc.tile_pool(name="epool", bufs=8) as epool, \
             tc.tile_pool(name="rpool", bufs=2) as rpool, \
             tc.tile_pool(name="outp", bufs=4) as outp:

            out_r = out.rearrange("(r p) n -> p r n", p=P)

            def bg_tile(name):
                return bgps.tile([P, 512], FP32, tag="bg", name=name)

            def qk_task(dst, w_sb, b_sb, r, half, on_scalar=False):
                ps = bg_tile(f"qk_{r}_{half}_{w_sb.name}")
                for kc in range(CT):
                    nc.tensor.matmul(
                        ps, w_sb[:, kc, P * r:P * r + P],
                        h_sb[:, kc, 512 * half:512 * half + 512],
                        start=(kc == 0), stop=(kc == CT - 1))
                dst_ap = dst[:, r, 512 * half:512 * half + 512]
                if on_scalar:
                    nc.scalar.add(dst_ap, ps, b_sb[:, r:r + 1])
                else:
                    nc.vector.tensor_scalar(dst_ap, ps, b_sb[:, r:r + 1],
                                            None, OP.add)

            def vt_task(t):
                ps = bg_tile(f"vt{t}")
                for kc in range(CT):
                    nc.tensor.matmul(ps, h_sb[:, kc, P * t:P * t + P],
                                     wv_sb[:, kc, :],
                                     start=(kc == 0), stop=(kc == CT - 1))
                nc.vector.tensor_copy(
                    vT_sb[:, t, :].rearrange("p (h c) -> p h c",
                                             c=P)[:, :, HD:P],
                    ps.rearrange("p (h c) -> p h c", c=HD))

            def proj01_task(r, half):
                hs = 512 * half
                ps = bg_tile(f"pjA_{r}_{half}")
                for kc in range(2):
                    nc.tensor.matmul(ps, pw_sb[:, kc, P * r:P * r + P],
                                     O_sb[:, kc, hs:hs + 512],
                                     start=(kc == 0), stop=(kc == 1))
                # P1x = (ps + pb) + x in one fused DVE op
                nc.vector.scalar_tensor_tensor(
                    P1x_sb[:, r, hs:hs + 512], ps, pb_sb[:, r:r + 1],
                    x_sb[:, r, hs:hs + 512], OP.add, OP.add)

            def proj23_task(r, half):
                hs = 512 * half
                ps = bg_tile(f"pjB_{r}_{half}")
                for kc in range(2, 4):
                    nc.tensor.matmul(ps, pw_sb[:, kc, P * r:P * r + P],
                                     O_sb[:, kc, hs:hs + 512],
                                     start=(kc == 2), stop=(kc == 3))
                o_st = outp.tile([P, 512], FP32, tag="ost",
                                 name=f"ost{r}_{half}")
                nc.vector.tensor_tensor(o_st, ps,
                                        P1x_sb[:, r, hs:hs + 512], OP.add)
                eng = nc.sync if (r + half) % 2 == 0 else nc.gpsimd
                eng.dma_start(out_r[:, r, hs:hs + 512], o_st)

            # drip schedule: exp-tile index (0..63) -> tasks. blocks are
            # half-major: b = 0..7 -> (pr = b % 4, half = b // 4); epilogue
            # of block b is emitted around tile 8b + 8 + LAG/2.
            drip = {
                0: [(vt_task, (0,)), (vt_task, (1,))],
                1: [(vt_task, (2,)), (vt_task, (3,))],
                2: [(vt_task, (4,)), (vt_task, (5,))],
                3: [(vt_task, (6,)), (vt_task, (7,))],
                4: [(qk_task, (k_sb, wk_sb, bk_sb, 1, 0))],
                5: [(qk_task, (k_sb, wk_sb, bk_sb, 1, 1))],
                6: [(qk_task, (q_sb, wq_sb, bq_sb, 1, 0))],
                9: [(qk_task, (k_sb, wk_sb, bk_sb, 2, 0))],
                11: [(qk_task, (k_sb, wk_sb, bk_sb, 2, 1))],
                13: [(qk_task, (q_sb, wq_sb, bq_sb, 2, 0))],
                17: [(qk_task, (k_sb, wk_sb, bk_sb, 3, 0))],
                19: [(qk_task, (k_sb, wk_sb, bk_sb, 3, 1))],
                21: [(qk_task, (q_sb, wq_sb, bq_sb, 3, 0))],
                25: [(qk_task, (q_sb, wq_sb, bq_sb, 0, 1))],
                20: [(proj01_task, (0, 0))],
                22: [(proj01_task, (1, 0))],
                24: [(proj01_task, (2, 0))],
                26: [(proj01_task, (3, 0))],
                33: [(qk_task, (q_sb, wq_sb, bq_sb, 1, 1))],
                37: [(proj23_task, (0, 0))],
                39: [(proj23_task, (1, 0))],
                41: [(proj23_task, (2, 0))],
                43: [(proj23_task, (3, 0))],
                45: [(qk_task, (q_sb, wq_sb, bq_sb, 2, 1))],
                49: [(qk_task, (q_sb, wq_sb, bq_sb, 3, 1))],
                53: [(proj01_task, (0, 1))],
                55: [(proj01_task, (1, 1))],
                57: [(proj01_task, (2, 1))],
                59: [(proj01_task, (3, 1))],
            }

            # per-hi O accumulators: [128, 512] = 1 PSUM bank each, 3
            # rotating bufs so the next block's AV never waits on the
            # previous block's epilogue
            O_cur = {}
            o_i = [0]

            def emit_av(b, hi, t, E_t, j):
                pr, half = b % 4, b // 4
                if t == 0:
                    o_i[0] = (o_i[0] + 1) % 3
                    O_cur[hi] = opool.tile([P, 512], FP32,
                                           tag=f"o{o_i[0]}",
                                           name=f"o{b}_{hi}")
                h = 2 * pr + hi
                nc.tensor.matmul(
                    O_cur[hi],
                    vT_sb[:, t, P * h:P * h + P],
                    E_t[:, j, :],
                    start=(t == 0), stop=(t == MT - 1))

            def emit_epilogue(b, hi):
                pr, half = b % 4, b // 4
                hs = 512 * half
                O_hi = O_cur.pop(hi)
                Rh = rpool.tile([HD, 512], FP32, tag="rh", name=f"rh{b}_{hi}")
                # D is broadcast on PSUM rows 0:64 (partition offset 0, as
                # the fast reciprocal requires)
                nc.vector.reciprocal_approx_fast(Rh, O_hi[0:HD, :])
                nc.vector.tensor_tensor(
                    O_sb[HD * hi:HD * hi + HD, pr, hs:hs + 512],
                    O_hi[HD:P, :], Rh, OP.mult)

            pend = deque()

            def flush_unit():
                b, hi, t, E_t, j = pend.popleft()
                emit_av(b, hi, t, E_t, j)
                if t == MT - 1:
                    emit_epilogue(b, hi)

            s_i = [0]
            for ti in range(64):
                b, rem = ti // 8, ti % 8
                pr, half = b % 4, b // 4
                hi, u = rem // 4, rem % 4
                s_i[0] ^= 1
                S_t = spool.tile([P, 2, 512], FP32, tag=f"s{s_i[0]}",
                                 name=f"st{ti}")
                for j in range(2):
                    t = 2 * u + j
                    nc.tensor.matmul(
                        S_t[:, j, :],
                        k_sb[HD * hi:HD * hi + HD, pr, P * t:P * t + P],
                        q_sb[HD * hi:HD * hi + HD, pr,
                             512 * half:512 * half + 512],
                        start=True, stop=True)
                E_t = epool.tile([P, 2, 512], BF16, tag="e", name=f"et{ti}")
                nc.scalar.activation(E_t, S_t, AF.Exp)
                for j in range(2):
                    pend.append((b, hi, 2 * u + j, E_t, j))
                while len(pend) > LAG:
                    flush_unit()
                for fn, args in drip.pop(ti, ()):
                    fn(*args)
            while pend:
                flush_unit()
            assert not drip, f"undripped: {list(drip)}"

            # ---------------- tail: proj kc 2:4 for half 1 ----------------
            with nc.named_scope("proj_tail"):
                for r in range(CT):
                    proj23_task(r, 1)


_CACHE: dict = {}


def _build():
    if "nc" in _CACHE:
        return _CACHE["nc"]
    nc = bacc.Bacc("TRN2", target_bir_lowering=False, debug=False,
                   num_devices=NCORES)
    io = {
        "x": nc.dram_tensor("x", [C, NT], FP32, kind="ExternalInput").ap(),
        "wq": nc.dram_tensor("wq", [P, CT, C], BF16, kind="ExternalInput").ap(),
        "wk": nc.dram_tensor("wk", [P, CT, C], BF16, kind="ExternalInput").ap(),
        "wv8": nc.dram_tensor("wv8", [P, CT, C], mybir.dt.float8e4,
                              kind="ExternalInput").ap(),
        "pw": nc.dram_tensor("pw", [P, CT, C], BF16, kind="ExternalInput").ap(),
        "bq": nc.dram_tensor("bq", [C], FP32, kind="ExternalInput").ap(),
        "bk": nc.dram_tensor("bk", [C], FP32, kind="ExternalInput").ap(),
        "pb": nc.dram_tensor("pb", [C], FP32, kind="ExternalInput").ap(),
        "gg": nc.dram_tensor("gg", [C], FP32, kind="ExternalInput").ap(),
        "gb": nc.dram_tensor("gb", [C], FP32, kind="ExternalInput").ap(),
        "amat": nc.dram_tensor("amat", [P, NH], FP32, kind="ExternalInput").ap(),
        "imat": nc.dram_tensor("imat", [NH, P], FP32, kind="ExternalInput").ap(),
        "out": nc.dram_tensor("out", [C, NT], FP32, kind="ExternalOutput").ap(),
    }
    with tile.TileContext(nc) as tc:
        _emit(tc, io)
    nc.compile()
    _CACHE["nc"] = nc
    return nc


def _host_prep(inputs):
    x = np.ascontiguousarray(np.asarray(inputs["x"], dtype=np.float32))
    qkv_w = np.asarray(inputs["qkv_w"], dtype=np.float32)
    qkv_b = np.asarray(inputs["qkv_b"], dtype=np.float32)
    proj_w = np.asarray(inputs["proj_w"], dtype=np.float32)
    proj_b = np.asarray(inputs["proj_b"], dtype=np.float32)
    gn_scale = np.asarray(inputs["gn_scale"], dtype=np.float32)
    gn_bias = np.asarray(inputs["gn_bias"], dtype=np.float32)

    s = np.float32(1.0 / np.sqrt(HD))
    bf = ml_dtypes.bfloat16

    def pack_qk(w):
        # [p, kc, oc] = w[oc, 128*kc + p]
        return np.ascontiguousarray(
            w.reshape(C, CT, P).transpose(2, 1, 0)).astype(bf)

    shared = {
        "wq": pack_qk(qkv_w[0:C] * s),
        "wk": pack_qk(qkv_w[C:2 * C]),
        "wv8": np.ascontiguousarray(
            qkv_w[2 * C:3 * C].reshape(C, CT, P).transpose(2, 1, 0)
        ).astype(ml_dtypes.float8_e4m3),
        "pw": pack_qk(proj_w),
        "bq": (qkv_b[0:C] * s).astype(np.float32),
        "bk": qkv_b[C:2 * C].astype(np.float32),
        # v bias and proj bias folded: proj(o + b_v) = proj(o) + W_p b_v
        "pb": (proj_b + proj_w @ qkv_b[2 * C:3 * C]).astype(np.float32),
        "gg": gn_scale,
        "gb": gn_bias,
        "amat": (np.kron(np.eye(NH, dtype=np.float32),
                         np.ones((GSZ, 1), np.float32)) / GSZ),
        "imat": np.ascontiguousarray(np.kron(np.eye(NH, dtype=np.float32),
                                             np.ones((1, GSZ), np.float32))),
    }
    B = x.shape[0]
    in_maps = []
    for b in range(B):
        m = dict(shared)
        m["x"] = np.ascontiguousarray(x[b].reshape(C, NT))
        in_maps.append(m)
    return in_maps


def run(inputs, trace=False):
    nc = _build()
    in_maps = _host_prep(inputs)
    res = run_bass_kernel_spmd(nc, in_maps, list(range(NCORES)), trace=trace)
    out = np.stack([res.results[i]["out"] for i in range(NCORES)], axis=0)
    return out.reshape(len(in_maps), C, 32, 32), res


def kernel(**inputs) -> np.ndarray:
    out, _ = run(inputs, trace=False)
    return out.astype(np.float32)


# revision 24
# speedup vs baseline: 1.0420x; 1.0116x over previous
"""Trainium2 Bass kernel for nn_AttentionBlock (GroupNorm -> MHA -> proj + residual).

Contract: kernel(**inputs) takes the FULL unsharded inputs (as produced by
setup_inputs) and returns the FULL output [8, 512, 32, 32] float32.

Sharding: pure data-parallel over batch B=8 across the 8 NeuronCores; each core
processes one batch element end-to-end (no collectives needed).

Per-core layout / algorithm (B=1, C=512, N=H*W=1024, heads=8, head_dim=64):
  - All matmuls bf16 (fp8 DoubleRow was measured to trigger a hardware
    power throttle to half clock, netting ~0 gain while slowing neighbors).
  - GroupNorm(32 groups) in fp32; bn_stats/bn_aggr per 128-channel tile,
    batched group-combine + broadcast via tiny PE matmuls, DVE-only rsqrt.
    All elementwise on DVE (gpsimd tensor ops measured at ~14.7us per
    [128,1024] tile -- unusable).
  - Attention in "S^T" layout: S^T[m,n] = sum_c k[c,m] q[c,n], K=64 bf16.
    exp on ScalarE -> bf16 E tiles ([128,2,512], 64 ACTIVATEs).
    AV with lhsT = [ones(64) | v(64)] per head: PSUM rows 0:64 hold the
    softmax denominator broadcast (partition offset 0 so
    reciprocal_approx_fast can read it directly), rows 64:128 hold O.
  - Blocks are half-major: b=0..7 -> (pr=b%4, half=b//4); O accumulates per
    block in one PSUM tile [128, 2(hi), 512]. Epilogue = 1 reciprocal +
    2 tensor_tensor mults straight out of PSUM (no copies).
  - proj per (r, half) split kc 0:2 / 2:4; first half fuses x + pb via
    scalar_tensor_tensor, second half adds and streams the output DMA
    inside the attention stream (only the last half=1 quarter is tail).
  - v-bias folded into pb on host (pb_eff = proj_b + proj_w @ b_v); q scale
    folded into wq/bq.
  - Static drip schedule interleaves qkv/vt/proj matmuls into the attention
    stream keyed on exp-tile index; AV lags exp by LAG units (software
    pipeline) so the PE never waits on ScalarE.
"""

import numpy as np
import ml_dtypes

import concourse.bass as bass
import concourse.tile as tile
from concourse import bacc, mybir
from concourse.bass_utils import run_bass_kernel_spmd

FP32 = mybir.dt.float32
BF16 = mybir.dt.bfloat16
AF = mybir.ActivationFunctionType
OP = mybir.AluOpType

P = 128      # SBUF partitions
C = 512      # channels
NT = 1024    # spatial tokens (32*32)
CT = C // P  # channel tiles = 4
MT = NT // P # key tiles = 8
NH = 8       # heads
HD = 64      # head dim
NCORES = 8
GSZ = 16     # channels per group (512/32)

LAG = 6  # AV units behind exp


def _emit(tc: "tile.TileContext", io: dict):
    nc = tc.nc
    from collections import deque
    import contextlib
    ctx = contextlib.ExitStack()
    with ctx:
        pers = ctx.enter_context(tc.tile_pool(name="pers", bufs=1))
        sm = ctx.enter_context(tc.tile_pool(name="small", bufs=1))

        x, wq, wk, pw = io["x"], io["wq"], io["wk"], io["pw"]
        out = io["out"]

        # ---------------- input DMAs ----------------
        # ~150 GB/s per queue; x (2MB) + wq/wk (1MB) must all land before the
        # first S matmul, so balance ~1MB per queue and split wq/wk by
        # kc-halves so early contraction steps don't wait for the full
        # tensor. pw/pb ride late (first use ~40us in).
        x_r = x.rearrange("(r p) n -> p r n", p=P)
        x_sb = pers.tile([P, CT, NT], FP32, tag="x")
        wq_sb = pers.tile([P, CT, C], BF16, tag="wq")
        wk_sb = pers.tile([P, CT, C], BF16, tag="wk")
        wv_sb = pers.tile([P, CT, C], mybir.dt.float8e4, tag="wv")
        pw_sb = pers.tile([P, CT, C], BF16, tag="pw")

        def xch(r, hf):
            return x_sb[:, r, 512 * hf:512 * hf + 512], \
                   x_r[:, r, 512 * hf:512 * hf + 512]

        # sync: small GN tensors, x tile 0, wq halves, bias smalls, pw
        amat_sb = pers.tile([P, NH], FP32, tag="amat")
        nc.sync.dma_start(amat_sb, io["amat"])
        imat_sb = pers.tile([NH, P], FP32, tag="imat")
        nc.sync.dma_start(imat_sb, io["imat"])
        gg_sb = pers.tile([P, CT], FP32, tag="gg")
        nc.sync.dma_start(gg_sb, io["gg"].rearrange("(r p) -> p r", p=P))
        gb_sb = pers.tile([P, CT], FP32, tag="gb")
        nc.sync.dma_start(gb_sb, io["gb"].rearrange("(r p) -> p r", p=P))
        nc.sync.dma_start(x_sb[:, 0, :], x_r[:, 0, :])
        nc.sync.dma_start(wq_sb[:, 0:2, :], wq[:, 0:2, :])
        nc.sync.dma_start(wq_sb[:, 2:4, :], wq[:, 2:4, :])
        bq_sb = pers.tile([P, CT], FP32, tag="bq")
        nc.sync.dma_start(bq_sb, io["bq"].rearrange("(r p) -> p r", p=P))
        bk_sb = pers.tile([P, CT], FP32, tag="bk")
        nc.sync.dma_start(bk_sb, io["bk"].rearrange("(r p) -> p r", p=P))
        pb_sb = pers.tile([P, CT], FP32, tag="pb")
        nc.sync.dma_start(pb_sb, io["pb"].rearrange("(r p) -> p r", p=P))
        nc.sync.dma_start(pw_sb, pw)
        # gpsimd: wk kc0/1 first (first contraction steps), x1, x3 low half,
        # wk kc2/3
        nc.gpsimd.dma_start(wk_sb[:, 0:2, :], wk[:, 0:2, :])
        nc.gpsimd.dma_start(x_sb[:, 1, :], x_r[:, 1, :])
        nc.gpsimd.dma_start(*xch(3, 0))
        nc.gpsimd.dma_start(wk_sb[:, 2:4, :], wk[:, 2:4, :])
        # scalar: x2, x3 high half, wv fp8 (ablation: fp8 wv benign)
        nc.scalar.dma_start(x_sb[:, 2, :], x_r[:, 2, :])
        nc.scalar.dma_start(*xch(3, 1))
        nc.scalar.dma_start(wv_sb, io["wv8"])

        # preload the exp activation table while DMAs are in flight
        warm_sb = pers.tile([1, 1], FP32, tag="actwarm")
        nc.vector.memset(warm_sb, 0.0)
        nc.scalar.activation(warm_sb, warm_sb, AF.Exp)

        # persistent SBUF
        h_sb = pers.tile([P, CT, NT], BF16, tag="h")
        q_sb = pers.tile([P, CT, NT], BF16, tag="q")
        k_sb = pers.tile([P, CT, NT], BF16, tag="k")
        # vT per head block: cols 0:64 = ones (denominator), 64:128 = v
        vT_sb = pers.tile([P, MT, NH * P], BF16, tag="vT")
        O_sb = pers.tile([P, CT, NT], BF16, tag="O")
        P1x_sb = pers.tile([P, CT, NT], FP32, tag="p1x")

        nc.gpsimd.memset(
            vT_sb.rearrange("p t (h c) -> p t h c", c=P)[:, :, :, 0:HD], 1.0)

        # ---------------- GroupNorm (per-tile pipelined) ----------------
        # groups (16ch) never cross a 128-channel tile, so each tile's full
        # GN chain runs as soon as its x chunks land, overlapped with later
        # DMAs; the three upfront q/k accumulations consume h tiles as they
        # appear (each in its own head-scoped PSUM bank).
        with nc.named_scope("gn"), \
             tc.tile_pool(name="gnps", bufs=1, space="PSUM") as gnps, \
             tc.tile_pool(name="mrps", bufs=1, space="PSUM") as mrps, \
             tc.tile_pool(name="hqk", bufs=1, space="PSUM") as hqk:
            up_ps = [hqk.tile([P, 512], FP32, tag=f"up{i}", name=f"up{i}")
                     for i in range(3)]
            up_spec = [(wk_sb, 0), (wk_sb, 1), (wq_sb, 0)]
            arrival = [2, 0, 1, 3]  # x tile DMA arrival order
            for gi, r in enumerate(arrival):
                st = sm.tile([P, 2, 6], FP32, tag=f"bnstats{r}")
                nc.vector.bn_stats(st[:, 0, :], x_sb[:, r, 0:512])
                nc.vector.bn_stats(st[:, 1, :], x_sb[:, r, 512:1024])
                mv = sm.tile([P, 2], FP32, tag=f"mv{r}")
                nc.vector.bn_aggr(mv, st)
                st2 = sm.tile([P, 2], FP32, tag=f"st2_{r}")
                nc.vector.tensor_copy(st2[:, 0:1], mv[:, 0:1])
                nc.vector.tensor_tensor(st2[:, 1:2], mv[:, 0:1], mv[:, 0:1],
                                        OP.mult)
                nc.vector.tensor_tensor(st2[:, 1:2], st2[:, 1:2], mv[:, 1:2],
                                        OP.add)
                G = gnps.tile([NH, 2], FP32, tag="g", name=f"g{r}")
                nc.tensor.matmul(G, amat_sb, st2, start=True, stop=True)
                stg = sm.tile([NH, 2], FP32, tag=f"stg{r}")
                nc.vector.tensor_copy(stg, G)
                var = sm.tile([NH, 1], FP32, tag=f"var{r}")
                nc.vector.tensor_tensor(var, stg[:, 0:1], stg[:, 0:1],
                                        OP.mult)
                nc.vector.tensor_tensor(var, stg[:, 1:2], var, OP.subtract)
                nc.vector.tensor_scalar(var, var, 1e-5, None, OP.add)
                # rstd = sqrt(1/(var+eps)): fast DVE reciprocal + ScalarE sqrt
                yv = sm.tile([NH, 1], FP32, tag=f"y{r}")
                nc.vector.reciprocal_approx_fast(yv, var)
                nc.scalar.activation(stg[:, 1:2], yv, AF.Sqrt)
                MR = mrps.tile([P, 2], FP32, tag="mr", name=f"mr{r}")
                nc.tensor.matmul(MR, imat_sb, stg, start=True, stop=True)
                a_r = sm.tile([P, 1], FP32, tag=f"a{r}")
                nc.vector.tensor_tensor(a_r, MR[:, 1:2], gg_sb[:, r:r + 1],
                                        OP.mult)
                b_r = sm.tile([P, 1], FP32, tag=f"b{r}")
                nc.vector.tensor_tensor(b_r, MR[:, 0:1], a_r, OP.mult)
                nc.vector.tensor_tensor(b_r, gb_sb[:, r:r + 1], b_r,
                                        OP.subtract)
                if r % 2 == 0:
                    nc.scalar.activation(h_sb[:, r, :], x_sb[:, r, :],
                                         AF.Identity, bias=b_r, scale=a_r)
                else:
                    nc.vector.tensor_scalar(h_sb[:, r, :], x_sb[:, r, :],
                                            a_r, b_r, OP.mult, OP.add)
                # feed this h tile into the three upfront q/k accumulations
                for i, (w_sb, half) in enumerate(up_spec):
                    nc.tensor.matmul(
                        up_ps[i], w_sb[:, r, 0:P],
                        h_sb[:, r, 512 * half:512 * half + 512],
                        start=(gi == 0), stop=(gi == CT - 1))
            # bias copies on ScalarE (idle until the exp stream starts)
            nc.scalar.add(k_sb[:, 0, 0:512], up_ps[0], bk_sb[:, 0:1])
            nc.scalar.add(k_sb[:, 0, 512:1024], up_ps[1], bk_sb[:, 0:1])
            nc.scalar.add(q_sb[:, 0, 0:512], up_ps[2], bq_sb[:, 0:1])

        # ------------- qkv + attention -------------
        with nc.named_scope("qkv_attn"), \
             tc.tile_pool(name="spool", bufs=1, space="PSUM") as spool, \
             tc.tile_pool(name="opool", bufs=1, space="PSUM") as opool, \
             tc.tile_pool(name="bgps", bufs=1, space="PSUM") as bgps, \
             tc.tile_pool(name="epool", bufs=8) as epool, \
             tc.tile_pool(name="rpool", bufs=2) as rpool, \
             tc.tile_pool(name="outp", bufs=4) as outp:

            out_r = out.rearrange("(r p) n -> p r n", p=P)

            def bg_tile(name):
                return bgps.tile([P, 512], FP32, tag="bg", name=name)

            def qk_task(dst, w_sb, b_sb, r, half, on_scalar=False):
                ps = bg_tile(f"qk_{r}_{half}_{w_sb.name}")
                for kc in range(CT):
                    nc.tensor.matmul(
                        ps, w_sb[:, kc, P * r:P * r + P],
                        h_sb[:, kc, 512 * half:512 * half + 512],
                        start=(kc == 0), stop=(kc == CT - 1))
                dst_ap = dst[:, r, 512 * half:512 * half + 512]
                if on_scalar:
                    nc.scalar.add(dst_ap, ps, b_sb[:, r:r + 1])
                else:
                    nc.vector.tensor_scalar(dst_ap, ps, b_sb[:, r:r + 1],
                                            None, OP.add)

            def vt_task(t):
                ps = bg_tile(f"vt{t}")
                for kc in range(CT):
                    nc.tensor.matmul(ps, h_sb[:, kc, P * t:P * t + P],
                                     wv_sb[:, kc, :],
                                     start=(kc == 0), stop=(kc == CT - 1))
                nc.vector.tensor_copy(
                    vT_sb[:, t, :].rearrange("p (h c) -> p h c",
                                             c=P)[:, :, HD:P],
                    ps.rearrange("p (h c) -> p h c", c=HD))

            def projA_task(r, half):
                # kc 0..2 partial sums + x + pb folded in one fused DVE op
                hs = 512 * half
                ps = bg_tile(f"pjA_{r}_{half}")
                for kc in range(3):
                    nc.tensor.matmul(ps, pw_sb[:, kc, P * r:P * r + P],
                                     O_sb[:, kc, hs:hs + 512],
                                     start=(kc == 0), stop=(kc == 2))
                nc.vector.scalar_tensor_tensor(
                    P1x_sb[:, r, hs:hs + 512], ps, pb_sb[:, r:r + 1],
                    x_sb[:, r, hs:hs + 512], OP.add, OP.add)

            def projB_task(r, half):
                # final kc=3 matmul + residual combine + output DMA
                hs = 512 * half
                ps = bg_tile(f"pjB_{r}_{half}")
                nc.tensor.matmul(ps, pw_sb[:, 3, P * r:P * r + P],
                                 O_sb[:, 3, hs:hs + 512],
                                 start=True, stop=True)
                o_st = outp.tile([P, 512], FP32, tag="ost",
                                 name=f"ost{r}_{half}")
                nc.vector.tensor_tensor(o_st, ps,
                                        P1x_sb[:, r, hs:hs + 512], OP.add)
                eng = (nc.sync, nc.gpsimd, nc.scalar)[(2 * half + r) % 3]
                eng.dma_start(out_r[:, r, hs:hs + 512], o_st)

            def projA1_task(r):
                # half-1 kc 0,1 partials (available after block b5)
                ps = bg_tile(f"pjA1_{r}")
                for kc in range(2):
                    nc.tensor.matmul(ps, pw_sb[:, kc, P * r:P * r + P],
                                     O_sb[:, kc, 512:1024],
                                     start=(kc == 0), stop=(kc == 1))
                nc.vector.scalar_tensor_tensor(
                    P1x_sb[:, r, 512:1024], ps, pb_sb[:, r:r + 1],
                    x_sb[:, r, 512:1024], OP.add, OP.add)

            def projM1_task(r):
                # half-1 kc 2 partial folded into P1x (after block b6)
                ps = bg_tile(f"pjM1_{r}")
                nc.tensor.matmul(ps, pw_sb[:, 2, P * r:P * r + P],
                                 O_sb[:, 2, 512:1024],
                                 start=True, stop=True)
                nc.vector.tensor_tensor(P1x_sb[:, r, 512:1024], ps,
                                        P1x_sb[:, r, 512:1024], OP.add)

            # drip schedule: exp-tile index (0..63) -> tasks. blocks are
            # half-major: b = 0..7 -> (pr = b % 4, half = b // 4); epilogue
            # of block b is emitted around tile 8b + 11.
            drip = {
                0: [(vt_task, (0,)), (vt_task, (1,))],
                1: [(vt_task, (2,)), (vt_task, (3,))],
                2: [(vt_task, (4,)), (vt_task, (5,))],
                3: [(vt_task, (6,)), (vt_task, (7,))],
                4: [(qk_task, (k_sb, wk_sb, bk_sb, 1, 0))],
                5: [(qk_task, (k_sb, wk_sb, bk_sb, 1, 1))],
                6: [(qk_task, (q_sb, wq_sb, bq_sb, 1, 0))],
                9: [(qk_task, (k_sb, wk_sb, bk_sb, 2, 0))],
                11: [(qk_task, (k_sb, wk_sb, bk_sb, 2, 1))],
                13: [(qk_task, (q_sb, wq_sb, bq_sb, 2, 0))],
                17: [(qk_task, (k_sb, wk_sb, bk_sb, 3, 0))],
                19: [(qk_task, (k_sb, wk_sb, bk_sb, 3, 1))],
                21: [(qk_task, (q_sb, wq_sb, bq_sb, 3, 0))],
                25: [(qk_task, (q_sb, wq_sb, bq_sb, 0, 1))],
                28: [(projA_task, (0, 0))],
                30: [(projA_task, (1, 0))],
                32: [(projA_task, (2, 0))],
                34: [(projA_task, (3, 0))],
                33: [(qk_task, (q_sb, wq_sb, bq_sb, 1, 1))],
                37: [(projB_task, (0, 0))],
                39: [(projB_task, (1, 0))],
                41: [(projB_task, (2, 0))],
                43: [(projB_task, (3, 0))],
                45: [(qk_task, (q_sb, wq_sb, bq_sb, 2, 1))],
                49: [(qk_task, (q_sb, wq_sb, bq_sb, 3, 1))],
                51: [(projA1_task, (0,))],
                53: [(projA1_task, (1,))],
                55: [(projA1_task, (2,))],
                57: [(projA1_task, (3,))],
                59: [(projM1_task, (0,))],
                60: [(projM1_task, (1,))],
                61: [(projM1_task, (2,))],
                62: [(projM1_task, (3,))],
            }

            # per-hi O accumulators: [128, 512] = 1 PSUM bank each, 3
            # rotating bufs so the next block's AV never waits on the
            # previous block's epilogue
            O_cur = {}
            o_i = [0]

            def emit_av(b, hi, t, E_t, j):
                pr, half = b % 4, b // 4
                if t == 0:
                    o_i[0] = (o_i[0] + 1) % 3
                    O_cur[hi] = opool.tile([P, 512], FP32,
                                           tag=f"o{o_i[0]}",
                                           name=f"o{b}_{hi}")
                h = 2 * pr + hi
                nc.tensor.matmul(
                    O_cur[hi],
                    vT_sb[:, t, P * h:P * h + P],
                    E_t[:, j, :],
                    start=(t == 0), stop=(t == MT - 1))

            def emit_epilogue(b, hi):
                pr, half = b % 4, b // 4
                hs = 512 * half
                O_hi = O_cur.pop(hi)
                Rh = rpool.tile([HD, 512], FP32, tag="rh", name=f"rh{b}_{hi}")
                # D is broadcast on PSUM rows 0:64 (partition offset 0, as
                # the fast reciprocal requires)
                nc.vector.reciprocal_approx_fast(Rh, O_hi[0:HD, :])
                nc.vector.tensor_tensor(
                    O_sb[HD * hi:HD * hi + HD, pr, hs:hs + 512],
                    O_hi[HD:P, :], Rh, OP.mult)

            pend = deque()

            def flush_unit():
                b, hi, t, E_t, j = pend.popleft()
                emit_av(b, hi, t, E_t, j)
                if t == MT - 1:
                    emit_epilogue(b, hi)

            s_i = [0]
            for ti in range(64):
                b, rem = ti // 8, ti % 8
                pr, half = b % 4, b // 4
                hi, u = rem // 4, rem % 4
                s_i[0] ^= 1
                S_t = spool.tile([P, 2, 512], FP32, tag=f"s{s_i[0]}",
                                 name=f"st{ti}")
                for j in range(2):
                    t = 2 * u + j
                    nc.tensor.matmul(
                        S_t[:, j, :],
                        k_sb[HD * hi:HD * hi + HD, pr, P * t:P * t + P],
                        q_sb[HD * hi:HD * hi + HD, pr,
                             512 * half:512 * half + 512],
                        start=True, stop=True)
                E_t = epool.tile([P, 2, 512], BF16, tag="e", name=f"et{ti}")
                nc.scalar.activation(E_t, S_t, AF.Exp)
                for j in range(2):
                    pend.append((b, hi, 2 * u + j, E_t, j))
                lag = LAG if ti < 58 else 3
                while len(pend) > lag:
                    flush_unit()
                for fn, args in drip.pop(ti, ()):
                    fn(*args)
            while pend:
                flush_unit()
            assert not drip, f"undripped: {list(drip)}"

            # ---------------- tail: final kc=3 proj for half 1 ----------------
            with nc.named_scope("proj_tail"):
                for r in range(CT):
                    projB_task(r, 1)


_CACHE: dict = {}


def _build():
    if "nc" in _CACHE:
        return _CACHE["nc"]
    nc = bacc.Bacc("TRN2", target_bir_lowering=False, debug=False,
                   num_devices=NCORES)
    io = {
        "x": nc.dram_tensor("x", [C, NT], FP32, kind="ExternalInput").ap(),
        "wq": nc.dram_tensor("wq", [P, CT, C], BF16, kind="ExternalInput").ap(),
        "wk": nc.dram_tensor("wk", [P, CT, C], BF16, kind="ExternalInput").ap(),
        "wv8": nc.dram_tensor("wv8", [P, CT, C], mybir.dt.float8e4,
                              kind="ExternalInput").ap(),
        "pw": nc.dram_tensor("pw", [P, CT, C], BF16, kind="ExternalInput").ap(),
        "bq": nc.dram_tensor("bq", [C], FP32, kind="ExternalInput").ap(),
        "bk": nc.dram_tensor("bk", [C], FP32, kind="ExternalInput").ap(),
        "pb": nc.dram_tensor("pb", [C], FP32, kind="ExternalInput").ap(),
        "gg": nc.dram_tensor("gg", [C], FP32, kind="ExternalInput").ap(),
        "gb": nc.dram_tensor("gb", [C], FP32, kind="ExternalInput").ap(),
        "amat": nc.dram_tensor("amat", [P, NH], FP32, kind="ExternalInput").ap(),
        "imat": nc.dram_tensor("imat", [NH, P], FP32, kind="ExternalInput").ap(),
        "out": nc.dram_tensor("out", [C, NT], FP32, kind="ExternalOutput").ap(),
    }
    with tile.TileContext(nc) as tc:
        _emit(tc, io)
    nc.compile()
    _CACHE["nc"] = nc
    return nc


def _host_prep(inputs):
    x = np.ascontiguousarray(np.asarray(inputs["x"], dtype=np.float32))
    qkv_w = np.asarray(inputs["qkv_w"], dtype=np.float32)
    qkv_b = np.asarray(inputs["qkv_b"], dtype=np.float32)
    proj_w = np.asarray(inputs["proj_w"], dtype=np.float32)
    proj_b = np.asarray(inputs["proj_b"], dtype=np.float32)
    gn_scale = np.asarray(inputs["gn_scale"], dtype=np.float32)
    gn_bias = np.asarray(inputs["gn_bias"], dtype=np.float32)

    s = np.float32(1.0 / np.sqrt(HD))
    bf = ml_dtypes.bfloat16

    def pack_qk(w):
        # [p, kc, oc] = w[oc, 128*kc + p]
        return np.ascontiguousarray(
            w.reshape(C, CT, P).transpose(2, 1, 0)).astype(bf)

    shared = {
        "wq": pack_qk(qkv_w[0:C] * s),
        "wk": pack_qk(qkv_w[C:2 * C]),
        "wv8": np.ascontiguousarray(
            qkv_w[2 * C:3 * C].reshape(C, CT, P).transpose(2, 1, 0)
        ).astype(ml_dtypes.float8_e4m3),
        "pw": pack_qk(proj_w),
        "bq": (qkv_b[0:C] * s).astype(np.float32),
        "bk": qkv_b[C:2 * C].astype(np.float32),
        # v bias and proj bias folded: proj(o + b_v) = proj(o) + W_p b_v
        "pb": (proj_b + proj_w @ qkv_b[2 * C:3 * C]).astype(np.float32),
        "gg": gn_scale,
        "gb": gn_bias,
        "amat": (np.kron(np.eye(NH, dtype=np.float32),
                         np.ones((GSZ, 1), np.float32)) / GSZ),
        "imat": np.ascontiguousarray(np.kron(np.eye(NH, dtype=np.float32),
                                             np.ones((1, GSZ), np.float32))),
    }
    B = x.shape[0]
    in_maps = []
    for b in range(B):
        m = dict(shared)
        m["x"] = np.ascontiguousarray(x[b].reshape(C, NT))
        in_maps.append(m)
    return in_maps


def run(inputs, trace=False):
    nc = _build()
    in_maps = _host_prep(inputs)
    res = run_bass_kernel_spmd(nc, in_maps, list(range(NCORES)), trace=trace)
    out = np.stack([res.results[i]["out"] for i in range(NCORES)], axis=0)
    return out.reshape(len(in_maps), C, 32, 32), res


def kernel(**inputs) -> np.ndarray:
    out, _ = run(inputs, trace=False)
    return out.astype(np.float32)
